# revision 1
# baseline (speedup 1.0000x reference)
# Trainium2 Bass kernel for nn_BinLinearEval:
#   out[b, o] = (round(x @ W.T + bias) * sign >= 0) ? 1.0 : 0.0
#
# Math folding (exact because bias is integer-valued and sign in {-1,+1}):
#   out = 1  iff  sign*(dot + bias) >= -0.5
#       = 1  iff  dot' >= thr_o      where dot' = x @ (sign.T*W).T  (W' still
#         ternary) and thr_o = -sign_o*bias_o - 0.5.
#
# Precision: x is shipped as an e4m3 hi + e4m3 residual*64 pair (2 B/elem,
# same HBM bytes as fp16) and BOTH passes run as fp8 DoubleRow matmuls at
# 0.5 cycles/column - the PE stream is ~2x faster than the fp16 single-pass
# variant, which measured clock-throttled to ~2 GHz under a dense fp16 MM
# stream. Accuracy: ~1713 threshold flips of 16.7M (rel err ~0.0143 vs the
# 2e-2 gate; verified in fp64 emulation and stable because inputs and the
# accumulation order are deterministic).
#
# PE schedule: one 512-column group per block, two oc passes of 8
# DoubleRow chunk-steps each. Unique per-matmul LDWEIGHTS hide fully in
# the PE's pull-ahead window at the measured 216 ns/MM pace, and
# single-group blocks spread the early data needs so the startup supply
# deficit stalls less than coarser shared-weight blocks did.
#
# DMA: the entire per-core x (16 MB = 128 KB/partition) fits in SBUF, so
# every group DMA is issued up front, split across BOTH HWDGE rings
# (SP + ACT) and ordered by receipt NEED (completion receipts reach
# consumers at max(transfer-done, prev receipt + ~2.2 us) per queue).
# Output is the is_ge threshold emitted as fp8 (1.0/0.0 exact), 1 B/elem.

import os
from contextlib import ExitStack

import numpy as np
import ml_dtypes

BATCH, IN_F, OUT_F = 65536, 1024, 256
N_CORES = 8
B_CORE = BATCH // N_CORES  # 8192
P = 128
KC = IN_F // P             # 8 k-chunks of 128
NCH = KC                   # 8 DoubleRow chunk-steps: 4 hi + 4 lo, 256-contract each
OC = OUT_F // P            # 2 out-channel chunks
GRP = 512                  # batch tile / group size (= max matmul moving dim)
N_GROUPS = B_CORE // GRP   # 16
# single-group blocks: they spread the early data needs (group N+1 is
# needed a full oc-pass later than with multi-group blocks), and unique
# per-matmul LDWEIGHTS hide fully in the PE pull-ahead window at the
# port-bound 216 ns/MM pace
BLOCKS = [1] * N_GROUPS
assert sum(BLOCKS) == N_GROUPS
# receipt-schedule ring split: receipts reach consumers at
# max(transfer-done, prev receipt + ~2.2 us) per queue, so early groups
# are placed by receipt NEED order: g1,g2 are SP receipts 2,3 (land
# ~12.6/15.1 vs needed 13.9/15.6); g3,g5 are ACT receipts 3,4; thr is
# SP receipt 5 (first needed by the epilogue, which tolerates it).
# Both rings carry 8 MB of x; ACT also has g0 up front + 2 MB of outs.
SYNC_GROUPS = frozenset([1, 2, 4, 6, 8, 10, 12, 14])

_CACHE = {}


def _build():
    """Build (and cache) the Bass module. Returns the compiled nc."""
    if "nc" in _CACHE:
        return _CACHE["nc"]

    import concourse.bacc as bacc
    import concourse.mybir as mybir
    import concourse.tile as tile

    nc = bacc.Bacc(
        "TRN2",
        target_bir_lowering=False,
        debug=False,
        num_devices=N_CORES,
    )

    f32 = mybir.dt.float32
    f8 = mybir.dt.float8e4
    DR = mybir.MatmulPerfMode.DoubleRow

    # x8 chunk layout: [P, group, chunk(0:4 hi, 4:8 lo), j, GRP] where the
    # DoubleRow pair (chunk c, j) covers global k = (c%4)*256 + j*128 + p
    x8_d = nc.dram_tensor(
        "x8", [P, N_GROUPS, NCH, 2, GRP], f8, kind="ExternalInput"
    ).ap()
    w8_d = nc.dram_tensor("w8", [P, NCH, 2, OUT_F], f8, kind="ExternalInput").ap()
    thr_d = nc.dram_tensor("thr", [P, OC], f32, kind="ExternalInput").ap()
    out_d = nc.dram_tensor("out", [OC, P, B_CORE], f8, kind="ExternalOutput").ap()

    with tile.TileContext(nc) as tc, ExitStack() as ctx:
        const = ctx.enter_context(tc.tile_pool(name="const", bufs=1))
        io = ctx.enter_context(tc.tile_pool(name="io", bufs=1))
        outp = ctx.enter_context(tc.tile_pool(name="outp", bufs=1))
        psum = ctx.enter_context(tc.tile_pool(name="psum", bufs=8, space="PSUM"))

        # startup: w8 as ONE SP DMA (single receipt gates the warm-up),
        # group 0 split hi/lo on ACT (hi-half receipt gates the first
        # real matmul; lo half lands before chunk-step 4 needs it)
        w8_sb = const.tile([P, NCH, 2, OUT_F], f8)
        thr_sb = const.tile([P, OC], f32)
        tiles = {}
        xg0 = io.tile([P, NCH, 2, GRP], f8, name="xg0", bufs=1)
        tiles[0] = xg0
        nc.sync.dma_start(out=w8_sb, in_=w8_d)
        nc.scalar.dma_start(out=xg0[:, : NCH // 2], in_=x8_d[:, 0, : NCH // 2])
        nc.scalar.dma_start(out=xg0[:, NCH // 2 :], in_=x8_d[:, 0, NCH // 2 :])
        nc.scalar.dma_start(out=thr_sb, in_=thr_d)

        # PE warm-up, dependency-gated: these dummy DoubleRow matmuls read
        # the just-DMA'd w8 hi half as BOTH operands, so they cannot start
        # before the weights land (~9.8 us) and they keep the PE's DVFS
        # ramp burning until the first x operands arrive (~12.2 us). The
        # psum results are never read.
        for i in range(7):
            wp_ps = psum.tile([P, GRP], f32, name="ps")
            nc.tensor.matmul(
                wp_ps[:, :OUT_F],
                w8_sb[:, 0, :, :P],
                w8_sb[:, 0],
                start=True,
                stop=True,
                perf_mode=DR,
            )

        # the ENTIRE per-core x (16 MB = 128 KB/partition) fits in SBUF:
        # issue every group DMA up front into its own tile. Both rings
        # stream flat-out with no pool-recycle or lookahead dependencies,
        # and multi-us ambient bandwidth dips are absorbed by buffered
        # slack instead of stalling the PE.
        def issue(g):
            eng = nc.sync if g in SYNC_GROUPS else nc.scalar
            t = io.tile([P, NCH, 2, GRP], f8, name=f"x{g}", bufs=1)
            eng.dma_start(out=t, in_=x8_d[:, g])
            tiles[g] = t

        issue(1)
        issue(2)
        issue(4)
        for g in range(1, N_GROUPS):
            if g not in tiles:
                issue(g)

        blocks = []
        g0 = 0
        for b in BLOCKS:
            blocks.append(list(range(g0, g0 + b)))
            g0 += b

        for bi, blk in enumerate(blocks):
            for oc in range(OC):
                pss = [psum.tile([P, GRP], f32, name="ps") for _ in blk]
                for c in range(NCH):
                    lhsT = w8_sb[:, c, :, oc * P : (oc + 1) * P]
                    for j, g in enumerate(blk):
                        nc.tensor.matmul(
                            pss[j],
                            lhsT,
                            tiles[g][:, c],
                            start=(c == 0),
                            stop=(c == NCH - 1),
                            perf_mode=DR,
                        )
                # one-off out tiles (32 x 512 B/partition total): out-DMAs
                # queue on ACT behind its x transfers, so recycled buffers
                # would stall epilogues -> psum -> PE; unique tiles decouple
                ob = outp.tile(
                    [P, len(blk) * GRP], f8, name=f"ob{bi}_{oc}", bufs=1
                )
                for j, g in enumerate(blk):
                    nc.vector.tensor_scalar(
                        ob[:, j * GRP : (j + 1) * GRP],
                        pss[j],
                        thr_sb[:, oc : oc + 1],
                        None,
                        mybir.AluOpType.is_ge,
                    )
                nc.scalar.dma_start(
                    out=out_d[oc, :, blk[0] * GRP : (blk[-1] + 1) * GRP], in_=ob
                )

    nc.compile()
    _CACHE["nc"] = nc
    return nc


def _prep_inputs(x, weight, bias, sign):
    """Host-side prep: fold sign into weights, build thresholds, split x into
    an e4m3 hi + e4m3 residual*64 pair in DoubleRow-interleaved layout."""
    f8np = ml_dtypes.float8_e4m3fn
    x = np.asarray(x, dtype=np.float32)
    weight = np.asarray(weight, dtype=np.float32)
    bias = np.asarray(bias, dtype=np.float32)
    sign = np.asarray(sign, dtype=np.float32).reshape(1, OUT_F)

    wp = sign.T * weight                      # [OUT_F, IN_F], ternary
    thr = (-sign[0] * bias - np.float32(0.5)).astype(np.float32)  # [OUT_F]
    thr2 = np.ascontiguousarray(thr.reshape(OC, P).T)  # [P, OC]

    # weights: [P, chunk, j, OUT_F]; chunks 0:4 = W' (ternary, exact in
    # e4m3), 4:8 = W'/64 (+-2^-6, exact in e4m3)
    wT = wp.T  # [IN_F, OUT_F]
    whi = wT.reshape(NCH // 2, 2, P, OUT_F).transpose(2, 0, 1, 3)
    wlo = (wT * np.float32(1.0 / 64.0)).reshape(NCH // 2, 2, P, OUT_F).transpose(
        2, 0, 1, 3
    )
    w8 = np.ascontiguousarray(
        np.concatenate([whi, wlo], axis=1)
    ).astype(f8np)                            # [P, NCH, 2, OUT_F]

    xhi8 = x.astype(f8np)
    xlo8 = ((x - xhi8.astype(np.float32)) * np.float32(64.0)).astype(f8np)

    in_maps = []
    for c in range(N_CORES):
        sl = slice(c * B_CORE, (c + 1) * B_CORE)
        hi = xhi8[sl].reshape(N_GROUPS, GRP, NCH // 2, 2, P).transpose(
            4, 0, 2, 3, 1
        )                                      # [P, g, 4, 2, GRP]
        lo = xlo8[sl].reshape(N_GROUPS, GRP, NCH // 2, 2, P).transpose(
            4, 0, 2, 3, 1
        )
        x8 = np.ascontiguousarray(np.concatenate([hi, lo], axis=2))
        in_maps.append({"x8": x8, "w8": w8, "thr": thr2})
    return in_maps


def _assemble(results):
    """[core][OC, P, B_CORE] fp8 -> [BATCH, OUT_F] fp32"""
    full = np.concatenate(
        [
            np.asarray(r["out"])
            .view(ml_dtypes.float8_e4m3fn)
            .astype(np.float32)
            .reshape(OUT_F, B_CORE)
            for r in results
        ],
        axis=1,
    )  # [OUT_F, BATCH]
    return np.ascontiguousarray(full.T)


def run(x, weight, bias, sign, trace=False):
    """Run the kernel; returns (output, BassKernelResults)."""
    from concourse.bass_utils import run_bass_kernel_spmd

    if not trace:
        os.environ["BASS_NEVER_TRACE"] = "1"
    else:
        os.environ.pop("BASS_NEVER_TRACE", None)

    nc = _build()
    in_maps = _prep_inputs(x, weight, bias, sign)
    res = run_bass_kernel_spmd(
        nc,
        in_maps,
        core_ids=list(range(N_CORES)),
        trace=trace,
    )
    return _assemble(res.results), res


def kernel(x, weight, bias, sign):
    out, _ = run(x, weight, bias, sign, trace=False)
    return out



# revision 3
# speedup vs baseline: 1.1783x; 1.1783x over previous
# Trainium2 Bass kernel for nn_BinLinearEval:
#   out[b, o] = (round(x @ W.T + bias) * sign >= 0) ? 1.0 : 0.0
#
# Math folding (exact because bias is integer-valued and sign in {-1,+1}):
#   out = 1  iff  sign*(dot + bias) >= -0.5
#       = 1  iff  dot' >= thr_o      where dot' = x @ (sign.T*W).T  (W' still
#         ternary) and thr_o = -sign_o*bias_o - 0.5.
#
# Precision: x is shipped as an e4m3 hi + e4m3 residual*64 pair (2 B/elem)
# and BOTH passes run as fp8 DoubleRow matmuls. ~1700 threshold flips of
# 16.7M (rel err ~0.014 vs the 2e-2 gate).
#
# Measured facts this schedule is built on (NTFF traces):
#  - At 8-core load the chip sits in P0: PE clock ~2.0 GHz, so a DR FD=512
#    matmul stream paces at exactly 259 ns/MM (216 ns single-core). The
#    256-MM stream is a hard 66.3 us floor; LDWEIGHTS fully hides in the
#    pull-ahead window at any weight-reuse pattern, so no LDW amortization
#    is needed.
#  - The framework preamble ends ~6.4 us; first DMA bytes move ~8.7-9 us.
#    Both HWDGE rings share the 16 SDMA engines per 4KB packet, so each
#    ring sustains ~175 GB/s while both are busy (~350 aggregate = HBM cap).
#  - Receipts (sem>=16) land ~50 ns after transfer-done; what matters is
#    pure need-ordering of the two ring FIFOs.
# Schedule: warmup MMs run on memset tiles (no DMA dependency) from ~7 us
# so HAM un-throttles before real data lands; the first 3 groups and w8
# are split across both rings in need order; outs are merged per group
# (1 KB/partition) and alternate rings.

import os
from contextlib import ExitStack

import numpy as np
import ml_dtypes

BATCH, IN_F, OUT_F = 65536, 1024, 256
N_CORES = 8
B_CORE = BATCH // N_CORES  # 8192
P = 128
KC = IN_F // P             # 8 k-chunks of 128
NCH = KC                   # 8 DoubleRow chunk-steps: 4 hi + 4 lo, 256-contract each
OC = OUT_F // P            # 2 out-channel chunks
GRP = 512                  # batch tile (= max DR matmul moving dim / 2)
N_GROUPS = B_CORE // GRP   # 16
N_WARM = 8                 # dummy MMs spanning ~4 us of PE-busy before data

_CACHE = {}


def _build():
    """Build (and cache) the Bass module. Returns the compiled nc."""
    if "nc" in _CACHE:
        return _CACHE["nc"]

    import concourse.bacc as bacc
    import concourse.mybir as mybir
    import concourse.tile as tile

    nc = bacc.Bacc(
        "TRN2",
        target_bir_lowering=False,
        debug=False,
        num_devices=N_CORES,
    )

    f32 = mybir.dt.float32
    f8 = mybir.dt.float8e4
    DR = mybir.MatmulPerfMode.DoubleRow

    # x8 chunk layout: [P, group, chunk(0:4 hi, 4:8 lo), j, GRP] where the
    # DoubleRow pair (chunk c, j) covers global k = (c%4)*256 + j*128 + p
    x8_d = nc.dram_tensor(
        "x8", [P, N_GROUPS, NCH, 2, GRP], f8, kind="ExternalInput"
    ).ap()
    # weights split by oc so each half is one contiguous 2KB/partition DMA
    w8_d = nc.dram_tensor("w8", [P, OC, NCH, 2, P], f8, kind="ExternalInput").ap()
    thr_d = nc.dram_tensor("thr", [P, OC], f32, kind="ExternalInput").ap()
    out_d = nc.dram_tensor(
        "out", [P, N_GROUPS, OC, GRP], f8, kind="ExternalOutput"
    ).ap()

    with tile.TileContext(nc) as tc, ExitStack() as ctx:
        const = ctx.enter_context(tc.tile_pool(name="const", bufs=1))
        io = ctx.enter_context(tc.tile_pool(name="io", bufs=1))
        outp = ctx.enter_context(tc.tile_pool(name="outp", bufs=1))
        psum = ctx.enter_context(tc.tile_pool(name="psum", bufs=8, space="PSUM"))

        w8_sb = const.tile([P, OC, NCH, 2, P], f8)
        thr_sb = const.tile([P, OC], f32)
        warm_w = const.tile([P, 2, P], f8)
        warm_x = const.tile([P, 2, GRP], f8)

        xt = {}
        for g in range(N_GROUPS):
            xt[g] = io.tile([P, NCH, 2, GRP], f8, name=f"x{g}", bufs=1)

        # warmup operands come from memset, not DMA, so the PE can start
        # burning its HAM ramp right after the preamble barrier (~6.5 us)
        nc.vector.memset(warm_w, 0.25)
        nc.vector.memset(warm_x, 0.25)

        # ── DMA triggers, need-ordered per ring ──
        # both rings drain at ~175 GB/s each while busy; arrival ~= zip of
        # the two FIFOs. Need times (T0~11.9us): w8oc0 & g0hi at T0, g0lo
        # T0+1.0, thr T0+2.1, w8oc1 T0+2.4, g1 T0+4.6, g2 T0+8.7, then one
        # group per 4.14 us.
        H = NCH // 2
        # sync ring: w8oc0, g0lo, g1lo, g2lo, g3, g5, g7, g9, g11, g13, g15
        nc.sync.dma_start(out=w8_sb[:, 0], in_=w8_d[:, 0])
        nc.sync.dma_start(out=xt[0][:, H:], in_=x8_d[:, 0, H:])
        nc.sync.dma_start(out=xt[1][:, H:], in_=x8_d[:, 1, H:])
        nc.sync.dma_start(out=xt[2][:, H:], in_=x8_d[:, 2, H:])
        # scalar ring: g0hi, thr, w8oc1, g1hi, g2hi, g4, g6, g8, g10, g12, g14
        nc.scalar.dma_start(out=xt[0][:, :H], in_=x8_d[:, 0, :H])
        nc.scalar.dma_start(out=thr_sb, in_=thr_d)
        nc.scalar.dma_start(out=w8_sb[:, 1], in_=w8_d[:, 1])
        nc.scalar.dma_start(out=xt[1][:, :H], in_=x8_d[:, 1, :H])
        nc.scalar.dma_start(out=xt[2][:, :H], in_=x8_d[:, 2, :H])
        for g in range(3, N_GROUPS):
            eng = nc.sync if g % 2 else nc.scalar
            eng.dma_start(out=xt[g], in_=x8_d[:, g])

        # ── PE warmup: data-independent DR MMs at cold pace (~0.52 us
        # each at P0 K=4/8) spanning ~4 us so HAM reaches K=8/8 before the
        # first real matmul. psum never read; slots recycle into the pool.
        wps = [psum.tile([P, GRP], f32, name="ps") for _ in range(2)]
        for i in range(N_WARM):
            nc.tensor.matmul(
                wps[i % 2], warm_w, warm_x, start=True, stop=True, perf_mode=DR
            )

        # ── main stream: 16 groups x 2 oc-passes x 8 chunk-steps ──
        for g in range(N_GROUPS):
            ob = outp.tile([P, OC, GRP], f8, name=f"ob{g}", bufs=1)
            for oc in range(OC):
                ps = psum.tile([P, GRP], f32, name="ps")
                for c in range(NCH):
                    nc.tensor.matmul(
                        ps,
                        w8_sb[:, oc, c],
                        xt[g][:, c],
                        start=(c == 0),
                        stop=(c == NCH - 1),
                        perf_mode=DR,
                    )
                nc.vector.tensor_scalar(
                    ob[:, oc],
                    ps,
                    thr_sb[:, oc : oc + 1],
                    None,
                    mybir.AluOpType.is_ge,
                )
            eng = nc.sync if g % 2 else nc.scalar
            eng.dma_start(out=out_d[:, g], in_=ob)

    nc.compile()
    _CACHE["nc"] = nc
    return nc


def _prep_inputs(x, weight, bias, sign):
    """Host-side prep: fold sign into weights, build thresholds, split x into
    an e4m3 hi + e4m3 residual*64 pair in DoubleRow-interleaved layout."""
    f8np = ml_dtypes.float8_e4m3fn
    x = np.asarray(x, dtype=np.float32)
    weight = np.asarray(weight, dtype=np.float32)
    bias = np.asarray(bias, dtype=np.float32)
    sign = np.asarray(sign, dtype=np.float32).reshape(1, OUT_F)

    wp = sign.T * weight                      # [OUT_F, IN_F], ternary
    thr = (-sign[0] * bias - np.float32(0.5)).astype(np.float32)  # [OUT_F]
    thr2 = np.ascontiguousarray(thr.reshape(OC, P).T)  # [P, OC]

    # weights: [P, oc, chunk, j, 128]; chunks 0:4 = W' (ternary, exact in
    # e4m3), 4:8 = W'/64 (+-2^-6, exact in e4m3)
    wT = wp.T  # [IN_F, OUT_F]
    whi = wT.reshape(NCH // 2, 2, P, OUT_F).transpose(2, 0, 1, 3)
    wlo = (wT * np.float32(1.0 / 64.0)).reshape(NCH // 2, 2, P, OUT_F).transpose(
        2, 0, 1, 3
    )
    w8 = np.concatenate([whi, wlo], axis=1)   # [P, NCH, 2, OUT_F]
    w8 = np.ascontiguousarray(
        w8.reshape(P, NCH, 2, OC, P).transpose(0, 3, 1, 2, 4)
    ).astype(f8np)                            # [P, OC, NCH, 2, P]

    xhi8 = x.astype(f8np)
    xlo8 = ((x - xhi8.astype(np.float32)) * np.float32(64.0)).astype(f8np)

    in_maps = []
    for c in range(N_CORES):
        sl = slice(c * B_CORE, (c + 1) * B_CORE)
        hi = xhi8[sl].reshape(N_GROUPS, GRP, NCH // 2, 2, P).transpose(
            4, 0, 2, 3, 1
        )                                      # [P, g, 4, 2, GRP]
        lo = xlo8[sl].reshape(N_GROUPS, GRP, NCH // 2, 2, P).transpose(
            4, 0, 2, 3, 1
        )
        x8 = np.ascontiguousarray(np.concatenate([hi, lo], axis=2))
        in_maps.append({"x8": x8, "w8": w8, "thr": thr2})
    return in_maps


def _assemble(results):
    """[core][P, N_GROUPS, OC, GRP] fp8 -> [BATCH, OUT_F] fp32"""
    parts = []
    for r in results:
        a = (
            np.asarray(r["out"])
            .view(ml_dtypes.float8_e4m3fn)
            .astype(np.float32)
            .reshape(P, N_GROUPS, OC, GRP)
        )
        # out[b, o]: b = g*GRP + col, o = oc*P + p
        parts.append(
            a.transpose(1, 3, 2, 0).reshape(B_CORE, OUT_F)
        )
    return np.ascontiguousarray(np.concatenate(parts, axis=0))


def run(x, weight, bias, sign, trace=False):
    """Run the kernel; returns (output, BassKernelResults)."""
    from concourse.bass_utils import run_bass_kernel_spmd

    if not trace:
        os.environ["BASS_NEVER_TRACE"] = "1"
    else:
        os.environ.pop("BASS_NEVER_TRACE", None)

    nc = _build()
    in_maps = _prep_inputs(x, weight, bias, sign)
    res = run_bass_kernel_spmd(
        nc,
        in_maps,
        core_ids=list(range(N_CORES)),
        trace=trace,
    )
    return _assemble(res.results), res


def kernel(x, weight, bias, sign):
    out, _ = run(x, weight, bias, sign, trace=False)
    return out


# revision 8
# speedup vs baseline: 1.2533x; 1.0636x over previous
# Trainium2 Bass kernel for nn_BinLinearEval:
#   out[b, o] = (round(x @ W.T + bias) * sign >= 0) ? 1.0 : 0.0
#
# Math folding (exact because bias is integer-valued and sign in {-1,+1}):
#   out = 1  iff  sign*(dot + bias) >= -0.5
#       = 1  iff  dot' >= thr_o      where dot' = x @ (sign.T*W).T  (W' still
#         ternary) and thr_o = -sign_o*bias_o - 0.5.
#
# Precision: x is shipped as an e4m3 hi + e4m3 residual*64 pair (2 B/elem)
# and BOTH passes run as fp8 DoubleRow matmuls. ~1700 threshold flips of
# 16.7M (rel err ~0.014 vs the 2e-2 gate).
#
# Measured facts this schedule is built on (NTFF traces):
#  - At 8-core load the chip sits in P0: PE clock ~2.0 GHz, so a DR FD=512
#    matmul stream paces at exactly 259 ns/MM (216 ns single-core). The
#    256-MM stream is a hard 66.3 us floor; LDWEIGHTS fully hides in the
#    pull-ahead window at any weight-reuse pattern, so no LDW amortization
#    is needed.
#  - The framework preamble ends ~6.4 us; first DMA bytes move ~8.7-9 us.
#    Both HWDGE rings share the 16 SDMA engines per 4KB packet, so each
#    ring sustains ~175 GB/s while both are busy (~350 aggregate = HBM cap).
#  - Receipts (sem>=16) land ~50 ns after transfer-done; what matters is
#    pure need-ordering of the two ring FIFOs.
# Schedule: warmup MMs run on memset tiles (no DMA dependency) from ~7 us
# so HAM un-throttles before real data lands; the first 3 groups and w8
# are split across both rings in need order; outs are merged per group
# (1 KB/partition) and alternate rings.

import os
from contextlib import ExitStack

import numpy as np
import ml_dtypes

BATCH, IN_F, OUT_F = 65536, 1024, 256
N_CORES = 8
B_CORE = BATCH // N_CORES  # 8192
P = 128
KC = IN_F // P             # 8 k-chunks of 128
NCH = KC                   # 8 DoubleRow chunk-steps: 4 hi + 4 lo, 256-contract each
OC = OUT_F // P            # 2 out-channel chunks
GRP = 512                  # batch tile (= max DR matmul moving dim / 2)
N_GROUPS = B_CORE // GRP   # 16
N_WARM = 10                # dummy MMs spanning ~4.3 us of PE-busy before data

_CACHE = {}


def _build():
    """Build (and cache) the Bass module. Returns the compiled nc."""
    if "nc" in _CACHE:
        return _CACHE["nc"]

    import concourse.bacc as bacc
    import concourse.mybir as mybir
    import concourse.tile as tile

    nc = bacc.Bacc(
        "TRN2",
        target_bir_lowering=False,
        debug=False,
        num_devices=N_CORES,
    )

    f32 = mybir.dt.float32
    f8 = mybir.dt.float8e4
    DR = mybir.MatmulPerfMode.DoubleRow

    # x8 chunk layout: [P, group, chunk(0:4 hi, 4:8 lo), j, GRP] where the
    # DoubleRow pair (chunk c, j) covers global k = (c%4)*256 + j*128 + p
    x8_d = nc.dram_tensor(
        "x8", [P, N_GROUPS, NCH, 2, GRP], f8, kind="ExternalInput"
    ).ap()
    # weights split by oc so each half is one contiguous 2KB/partition DMA
    w8_d = nc.dram_tensor("w8", [P, OC, NCH, 2, P], f8, kind="ExternalInput").ap()
    thr_d = nc.dram_tensor("thr", [P, OC], f32, kind="ExternalInput").ap()
    out_d = nc.dram_tensor(
        "out", [P, N_GROUPS, OC, GRP], f8, kind="ExternalOutput"
    ).ap()

    with tile.TileContext(nc) as tc, ExitStack() as ctx:
        const = ctx.enter_context(tc.tile_pool(name="const", bufs=1))
        io = ctx.enter_context(tc.tile_pool(name="io", bufs=1))
        outp = ctx.enter_context(tc.tile_pool(name="outp", bufs=1))
        psum = ctx.enter_context(tc.tile_pool(name="psum", bufs=8, space="PSUM"))

        w8_sb = const.tile([P, OC, NCH, 2, P], f8)
        thr_sb = const.tile([P, OC], f32)
        warm_x = const.tile([P, 2, GRP], f8)

        xt = {}
        for g in range(N_GROUPS):
            xt[g] = io.tile([P, NCH, 2, GRP], f8, name=f"x{g}", bufs=1)

        # warmup operand comes from one memset, not DMA, so the PE can
        # start burning its HAM ramp right after the preamble barrier
        nc.vector.memset(warm_x, 0.25)

        # ── DMA triggers ──
        # Both HWDGE rings share the 16 SDMA engines per-packet, ~185 GB/s
        # each while both are busy. Every group is split hi/lo across the
        # two rings in lockstep so group k completes ~2.7k us after the
        # first bytes — always ahead of the PE's 3.46 us/group consumption.
        # thr's 8-byte-per-partition descriptors would waste ring turns at
        # the worst time, so it rides the idle gpsimd SWDGE path instead.
        H = NCH // 2
        nc.sync.dma_start(out=w8_sb[:, 0], in_=w8_d[:, 0])
        nc.scalar.dma_start(out=xt[0][:, :H], in_=x8_d[:, 0, :H])
        nc.sync.dma_start(out=xt[0][:, H:], in_=x8_d[:, 0, H:])
        nc.scalar.dma_start(out=w8_sb[:, 1], in_=w8_d[:, 1])
        for g in range(1, N_GROUPS):
            nc.scalar.dma_start(out=xt[g][:, :H], in_=x8_d[:, g, :H])
            nc.sync.dma_start(out=xt[g][:, H:], in_=x8_d[:, g, H:])
            if g == 2:
                # thr's 8B-per-partition descriptors waste ring turns, so
                # it goes late: only the first epilogue (~psum-slack bound,
                # ~24us) needs it
                nc.sync.dma_start(out=thr_sb, in_=thr_d)

        # ── PE warmup: data-independent DR MMs at cold pace (~0.43-0.52
        # us each) spanning ~4.3 us so HAM reaches K=8/8 before the first
        # real matmul. psum never read; slots recycle into the pool.
        wps = [psum.tile([P, GRP], f32, name="ps") for _ in range(2)]
        for i in range(N_WARM):
            nc.tensor.matmul(
                wps[i % 2], warm_x[:, :, :P], warm_x,
                start=True, stop=True, perf_mode=DR,
            )

        # ── main stream: 16 groups x 2 oc-passes x 8 chunk-steps ──
        # outs for g0..g14 ride gpsimd SWDGE (latency-tolerant, keeps the
        # HWDGE rings clean for x); the last group's out is split per-oc
        # across the two then-idle HWDGE rings to minimize the tail receipt.
        for g in range(N_GROUPS):
            ob = outp.tile([P, OC, GRP], f8, name=f"ob{g}", bufs=1)
            last = g == N_GROUPS - 1
            for oc in range(OC):
                ps = psum.tile([P, GRP], f32, name="ps")
                for c in range(NCH):
                    nc.tensor.matmul(
                        ps,
                        w8_sb[:, oc, c],
                        xt[g][:, c],
                        start=(c == 0),
                        stop=(c == NCH - 1),
                        perf_mode=DR,
                    )
                nc.vector.tensor_scalar(
                    ob[:, oc],
                    ps,
                    thr_sb[:, oc : oc + 1],
                    None,
                    mybir.AluOpType.is_ge,
                )
                if last:
                    eng = nc.sync if oc == 0 else nc.scalar
                    eng.dma_start(out=out_d[:, g, oc], in_=ob[:, oc])
            if not last:
                eng = nc.sync if g % 2 else nc.scalar
                eng.dma_start(out=out_d[:, g], in_=ob)

    nc.compile()
    _CACHE["nc"] = nc
    return nc


def _prep_inputs(x, weight, bias, sign):
    """Host-side prep: fold sign into weights, build thresholds, split x into
    an e4m3 hi + e4m3 residual*64 pair in DoubleRow-interleaved layout."""
    f8np = ml_dtypes.float8_e4m3fn
    x = np.asarray(x, dtype=np.float32)
    weight = np.asarray(weight, dtype=np.float32)
    bias = np.asarray(bias, dtype=np.float32)
    sign = np.asarray(sign, dtype=np.float32).reshape(1, OUT_F)

    wp = sign.T * weight                      # [OUT_F, IN_F], ternary
    thr = (-sign[0] * bias - np.float32(0.5)).astype(np.float32)  # [OUT_F]
    thr2 = np.ascontiguousarray(thr.reshape(OC, P).T)  # [P, OC]

    # weights: [P, oc, chunk, j, 128]; chunks 0:4 = W' (ternary, exact in
    # e4m3), 4:8 = W'/64 (+-2^-6, exact in e4m3)
    wT = wp.T  # [IN_F, OUT_F]
    whi = wT.reshape(NCH // 2, 2, P, OUT_F).transpose(2, 0, 1, 3)
    wlo = (wT * np.float32(1.0 / 64.0)).reshape(NCH // 2, 2, P, OUT_F).transpose(
        2, 0, 1, 3
    )
    w8 = np.concatenate([whi, wlo], axis=1)   # [P, NCH, 2, OUT_F]
    w8 = np.ascontiguousarray(
        w8.reshape(P, NCH, 2, OC, P).transpose(0, 3, 1, 2, 4)
    ).astype(f8np)                            # [P, OC, NCH, 2, P]

    xhi8 = x.astype(f8np)
    xlo8 = ((x - xhi8.astype(np.float32)) * np.float32(64.0)).astype(f8np)

    in_maps = []
    for c in range(N_CORES):
        sl = slice(c * B_CORE, (c + 1) * B_CORE)
        hi = xhi8[sl].reshape(N_GROUPS, GRP, NCH // 2, 2, P).transpose(
            4, 0, 2, 3, 1
        )                                      # [P, g, 4, 2, GRP]
        lo = xlo8[sl].reshape(N_GROUPS, GRP, NCH // 2, 2, P).transpose(
            4, 0, 2, 3, 1
        )
        x8 = np.ascontiguousarray(np.concatenate([hi, lo], axis=2))
        in_maps.append({"x8": x8, "w8": w8, "thr": thr2})
    return in_maps


def _assemble(results):
    """[core][P, N_GROUPS, OC, GRP] fp8 -> [BATCH, OUT_F] fp32"""
    parts = []
    for r in results:
        a = (
            np.asarray(r["out"])
            .view(ml_dtypes.float8_e4m3fn)
            .astype(np.float32)
            .reshape(P, N_GROUPS, OC, GRP)
        )
        # out[b, o]: b = g*GRP + col, o = oc*P + p
        parts.append(
            a.transpose(1, 3, 2, 0).reshape(B_CORE, OUT_F)
        )
    return np.ascontiguousarray(np.concatenate(parts, axis=0))


def run(x, weight, bias, sign, trace=False):
    """Run the kernel; returns (output, BassKernelResults)."""
    from concourse.bass_utils import run_bass_kernel_spmd

    if not trace:
        os.environ["BASS_NEVER_TRACE"] = "1"
    else:
        os.environ.pop("BASS_NEVER_TRACE", None)

    nc = _build()
    in_maps = _prep_inputs(x, weight, bias, sign)
    res = run_bass_kernel_spmd(
        nc,
        in_maps,
        core_ids=list(range(N_CORES)),
        trace=trace,
    )
    return _assemble(res.results), res


def kernel(x, weight, bias, sign):
    out, _ = run(x, weight, bias, sign, trace=False)
    return out
